# revision 26
# baseline (speedup 1.0000x reference)
"""Trainium2 Bass kernel for nn_BaseLayerGate (MoE balanced routing).

8 NeuronCores, data-parallel over tokens:
  - Each core owns a 2048-token shard. Affinity matmul aff^T = centT.T @ featsT
    on the tensor engine (fp32): col-major aff^T [128 (2 slots x 64 experts), 2048].
  - Sinkhorn (10 iters) in reciprocal-potential form:
      R_sum[n]  = sum_se E0[n,se] * V[se]      (PE matvec, V = slot-masked 1/C_sum)
      C_sum[se] = sum_n  E0[n,se] * W[n]       (PE matvec accum, W = 1/R_sum)
    The token-direction sum is global: per-expert partials are exchanged with an
    AllGather of a [1,128] row (PE-transposed so both exchange DMAs are
    contiguous), summed on-chip. 10 R-steps, 9 C-steps/exchanges (the 10th
    C-step is a uniform per-column shift and cannot change top-k ordering).
  - Z^T = aff^T - ln(R_sum) broadcast (ACT Ln + one Newton step for the LUT),
    per-column ordering of Z equals the reference's final ordering.
"""

import numpy as np

import concourse.bass as bass
from concourse import mybir
from concourse.bass_utils import run_bass_kernel_spmd

N_CORES = 8
N = 16384
D = 1024
KSLOT = 2
E = 64
SE = KSLOT * E
CAP = N // E
TOK = N // N_CORES
ITERS = 10

F32 = mybir.dt.float32

# ---- semaphore schedule ----------------------------------------------------
# in_sem: centT(8) + v0 + ident + ones = 11 transfers; fsem[k]: featsT chunk k
# out_sem: afft + zt outputs
# dma_sem (all x16): exchange cc_in(it) -> 2it+1, gath(it) -> 2it+2; rflat +2
D_EXCH_END = 2 * (ITERS - 1)                 # 18
D_RFLAT = D_EXCH_END + 2                     # 20
# pe_sem: 1 aff | +16 transposes -> 17 | per iter: R -> 17+2it+1, C -> 17+2it+2
def P_R(it):
    return 17 + 2 * it + 1
def P_C(it):
    return 17 + 2 * it + 2
P_LAST_R = P_R(ITERS - 1)                     # 36
P_ZB = P_LAST_R + 3                           # 39
# act_sem: 1 aff-copy | 2 exp | e0tm copy t -> t+3 (-> 18) | Ln | texp | rT x2
A_E0TM = 18
A_LN = A_E0TM + 1                             # 19
A_TEXP = A_LN + 1                             # 20
A_RT = A_TEXP + 2                             # 22
# dve_sem: 1 affT copy | per it<9: W=2+2it, VU=3+2it | u, rl2, sub
def V_W(it):
    return 3 * it + 1
def V_EX(it):
    return 3 * it + 2
def V_VU(it):
    return 3 * it + 3
V_U = 3 * (ITERS - 1) + 1                     # 28
V_RL2 = V_U + 1                               # 29
V_SUB = V_RL2 + 1                             # 30


def _build_nc():
    nc = bass.Bass()

    featsT_in = nc.declare_dram_parameter("featsT", [D, TOK], F32, isOutput=False)
    centT_in = nc.declare_dram_parameter("centT", [D, SE], F32, isOutput=False)
    v0_in = nc.declare_dram_parameter("v0", [SE, 2], F32, isOutput=False)
    ident_in = nc.declare_dram_parameter("ident", [128, 128], F32, isOutput=False)
    ones_in = nc.declare_dram_parameter("ones", [1, 64], F32, isOutput=False)
    onesc_in = nc.declare_dram_parameter("onesc", [128, 1], F32, isOutput=False)

    zt_out = nc.declare_dram_parameter("zt", [SE, TOK], F32, isOutput=True)
    aff_out = nc.declare_dram_parameter("afft", [SE, TOK], F32, isOutput=True)

    cc_in = nc.dram_tensor("cc_in", [SE, 1], F32)
    cc_out = nc.dram_tensor("cc_out", [SE, 1], F32, addr_space="Shared")

    core_ids = list(range(N_CORES))

    from contextlib import ExitStack
    es = ExitStack()
    featsT_sb = es.enter_context(nc.sbuf_tensor("featsT_sb", [128, 8, TOK], F32))
    centT_sb = es.enter_context(nc.sbuf_tensor("centT_sb", [128, 8, SE], F32))
    affT_sb = es.enter_context(nc.sbuf_tensor("affT_sb", [128, TOK], F32))
    e0t_sb = es.enter_context(nc.sbuf_tensor("e0t_sb", [128, TOK], F32))
    e0tm_sb = es.enter_context(nc.sbuf_tensor("e0tm_sb", [128, 16, 128], F32))
    ident_sb = es.enter_context(nc.sbuf_tensor("ident_sb", [128, 128], F32))
    v_sb = es.enter_context(nc.sbuf_tensor("v_sb", [128, 2], F32))
    w_sb = es.enter_context(nc.sbuf_tensor("w_sb", [128, 16, 2], F32))
    cpart_sb = es.enter_context(nc.sbuf_tensor("cpart_sb", [128, 1], F32))
    crow_sb = es.enter_context(nc.sbuf_tensor("crow_sb", [1, SE], F32))
    gath_sb = es.enter_context(nc.sbuf_tensor("gath_sb", [128, SE], F32))
    csum_sb = es.enter_context(nc.sbuf_tensor("csum_sb", [128, 1], F32))
    rlog_sb = es.enter_context(nc.sbuf_tensor("rlog_sb", [128, 16, 2], F32))
    texp_sb = es.enter_context(nc.sbuf_tensor("texp_sb", [128, 32], F32))
    u_sb = es.enter_context(nc.sbuf_tensor("u_sb", [128, 32], F32))
    rlog2_sb = es.enter_context(nc.sbuf_tensor("rlog2_sb", [128, 16, 2], F32))
    rt_sb = es.enter_context(nc.sbuf_tensor("rt_sb", [16, 2, 128], F32))
    rflat0_sb = es.enter_context(nc.sbuf_tensor("rflat0_sb", [1, TOK], F32))
    rflat1_sb = es.enter_context(nc.sbuf_tensor("rflat1_sb", [1, TOK], F32))
    ones_sb = es.enter_context(nc.sbuf_tensor("ones_sb", [1, 64], F32))
    onesc_sb = es.enter_context(nc.sbuf_tensor("onesc_sb", [128, 1], F32))
    zt_sb = es.enter_context(nc.sbuf_tensor("zt_sb", [128, TOK], F32))
    ps_aff = es.enter_context(nc.psum_tensor("ps_aff", [128, TOK], F32))
    ps_tp = es.enter_context(nc.psum_tensor("ps_tp", [128, 512], F32))
    ps_r = es.enter_context(nc.psum_tensor("ps_r", [128, 512], F32))
    ps_c = es.enter_context(nc.psum_tensor("ps_c", [128, 512], F32))
    block = es.enter_context(nc.Block())
    dma_sem = es.enter_context(nc.semaphore("dma_sem"))
    in_sem = es.enter_context(nc.semaphore("in_sem"))
    out_sem = es.enter_context(nc.semaphore("out_sem"))
    fsems = [es.enter_context(nc.semaphore(f"fsem{k}")) for k in range(8)]
    pe_sem = es.enter_context(nc.semaphore("pe_sem"))
    act_sem = es.enter_context(nc.semaphore("act_sem"))
    dve_sem = es.enter_context(nc.semaphore("dve_sem"))
    cc_sem = es.enter_context(nc.semaphore("cc_sem"))
    with es:
        # ---------------- sync engine: all DMA ----------------
        @block.sync
        def _(eng):
            for k in range(8):
                eng.dma_start(
                    out=centT_sb[:, k, :], in_=centT_in[128 * k : 128 * (k + 1), :]
                ).then_inc(in_sem, 16)
            eng.dma_start(out=v_sb[:], in_=v0_in[:]).then_inc(in_sem, 16)
            eng.dma_start(out=ident_sb[:], in_=ident_in[:]).then_inc(in_sem, 16)
            eng.dma_start(out=ones_sb[:], in_=ones_in[:]).then_inc(in_sem, 16)
            eng.dma_start(out=onesc_sb[:], in_=onesc_in[:]).then_inc(in_sem, 16)
            for k in range(8):
                eng.dma_start(
                    out=featsT_sb[:, k, :], in_=featsT_in[128 * k : 128 * (k + 1), :]
                ).then_inc(fsems[k], 16)

            eng.wait_ge(act_sem, 1)
            eng.dma_start(out=aff_out[:], in_=affT_sb[:]).then_inc(out_sem, 16)

            for it in range(ITERS - 1):
                eng.wait_ge(dve_sem, V_EX(it))
                eng.dma_start(out=cc_in[:], in_=cpart_sb[:]).then_inc(dma_sem, 16)
                eng.wait_ge(cc_sem, it + 1)
                eng.dma_start(out=csum_sb[:], in_=cc_out[:]).then_inc(dma_sem, 16)

            eng.wait_ge(act_sem, A_RT)
            for s in range(2):
                dsts = (rflat0_sb if s == 0 else rflat1_sb).ap()[0:1]
                dsts = dsts.rearrange("o (t p) -> o t p", p=128)
                eng.dma_start(out=dsts, in_=rt_sb[:, s, :]).then_inc(dma_sem, 16)

            eng.wait_ge(dve_sem, V_SUB)
            eng.dma_start(out=zt_out[:], in_=zt_sb[:]).then_inc(out_sem, 16)
            eng.wait_ge(out_sem, 32)
            eng.wait_ge(dma_sem, 16 * D_RFLAT)

        # ---------------- tensor engine ----------------
        @block.tensor
        def _(eng):
            eng.wait_ge(in_sem, 16 * 12)
            for k in range(8):
                eng.wait_ge(fsems[k], 16)
                for n in range(4):
                    mm = eng.matmul(
                        ps_aff[:, 512 * n : 512 * (n + 1)],
                        centT_sb[:, k, :],
                        featsT_sb[:, k, 512 * n : 512 * (n + 1)],
                        start=(k == 0),
                        stop=(k == 7),
                    )
            mm.then_inc(pe_sem, 1)

            for t in range(16):
                eng.wait_ge(act_sem, 2 + t)  # exp done (t=0) / ps_tp freed (t>0)
                eng.transpose(
                    ps_tp[:, 0:128], e0t_sb[:, 128 * t : 128 * (t + 1)], ident_sb[:]
                ).then_inc(pe_sem, 1)

            for it in range(ITERS):
                if it > 0:
                    eng.wait_ge(dve_sem, V_VU(it - 1))
                for t in range(16):
                    mm = eng.matmul(
                        ps_r[:, 2 * t : 2 * (t + 1)],
                        e0t_sb[:, 128 * t : 128 * (t + 1)],
                        v_sb[:],
                        start=True,
                        stop=True,
                    )
                mm.then_inc(pe_sem, 1)

                if it < ITERS - 1:
                    if it == 0:
                        eng.wait_ge(act_sem, A_E0TM)  # all e0tm copies landed
                    eng.wait_ge(dve_sem, V_W(it))
                    for t in range(16):
                        mm = eng.matmul(
                            ps_c[:, 0:2],
                            e0tm_sb[:, t, :],
                            w_sb[:, t, :],
                            start=(t == 0),
                            stop=(t == 15),
                        )
                    mm.then_inc(pe_sem, 1)

            eng.wait_ge(dve_sem, V_RL2)
            for s in range(2):
                eng.transpose(ps_tp[0:16, 0:128], rlog2_sb[:, :, s], ident_sb[:]).then_inc(pe_sem, 1)
                eng.wait_ge(act_sem, A_TEXP + 1 + s)  # ACT copied ps_tp before reuse

            eng.wait_ge(dma_sem, 16 * D_RFLAT)
            for s in range(2):
                rsrc = rflat0_sb if s == 0 else rflat1_sb
                for n in range(4):
                    mm = eng.matmul(
                        ps_aff[64 * s : 64 * (s + 1), 512 * n : 512 * (n + 1)],
                        ones_sb[0:1, :],
                        rsrc[0:1, 512 * n : 512 * (n + 1)],
                        start=True,
                        stop=True,
                    )
            mm.then_inc(pe_sem, 1)

        # ---------------- scalar (ACT) engine ----------------
        @block.scalar
        def _(eng):
            eng.wait_ge(pe_sem, 1)
            eng.activation(affT_sb[:], ps_aff[:, 0:TOK], mybir.ActivationFunctionType.Copy).then_inc(act_sem, 1)
            eng.wait_ge(act_sem, 1)
            eng.activation(e0t_sb[:], affT_sb[:], mybir.ActivationFunctionType.Exp).then_inc(act_sem, 1)
            for t in range(16):
                eng.wait_ge(pe_sem, 2 + t)
                eng.activation(
                    e0tm_sb[:, t, :], ps_tp[:, 0:128], mybir.ActivationFunctionType.Copy
                ).then_inc(act_sem, 1)
            eng.wait_ge(pe_sem, P_LAST_R)
            eng.activation(
                rlog_sb.ap().rearrange("p t s -> p (t s)"),
                ps_r[:, 0:32],
                mybir.ActivationFunctionType.Ln,
            ).then_inc(act_sem, 1)
            eng.wait_ge(act_sem, A_LN)
            eng.activation(
                texp_sb[:],
                rlog_sb.ap().rearrange("p t s -> p (t s)"),
                mybir.ActivationFunctionType.Exp,
                scale=-1.0,
            ).then_inc(act_sem, 1)
            for s in range(2):
                eng.wait_ge(pe_sem, P_LAST_R + 1 + s)
                eng.activation(rt_sb[:, s, :], ps_tp[0:16, 0:128], mybir.ActivationFunctionType.Copy).then_inc(act_sem, 1)


        # ---------------- vector (DVE) engine ----------------
        @block.vector
        def _(eng):
            for it in range(ITERS - 1):
                eng.wait_ge(pe_sem, P_R(it))
                eng.reciprocal(w_sb.ap().rearrange("p t s -> p (t s)"), ps_r[:, 0:32]).then_inc(dve_sem, 1)
                eng.wait_ge(pe_sem, P_C(it))
                eng.tensor_copy(cpart_sb[0:64, :], ps_c[0:64, 0:1])
                eng.tensor_copy(cpart_sb[64:128, :], ps_c[64:128, 1:2]).then_inc(dve_sem, 1)
                eng.wait_ge(dma_sem, 16 * (2 * it + 2))
                eng.reciprocal(v_sb[0:64, 0:1], csum_sb[0:64, :])
                eng.reciprocal(v_sb[64:128, 1:2], csum_sb[64:128, :]).then_inc(dve_sem, 1)
            eng.wait_ge(pe_sem, P_LAST_R)
            eng.wait_ge(act_sem, A_TEXP)
            eng.tensor_mul(u_sb[:], ps_r[:, 0:32], texp_sb[:]).then_inc(dve_sem, 1)
            eng.wait_ge(dve_sem, V_U)
            eng.scalar_tensor_tensor(
                rlog2_sb.ap().rearrange("p t s -> p (t s)"),
                u_sb[:],
                1.0,
                rlog_sb.ap().rearrange("p t s -> p (t s)"),
                op0=mybir.AluOpType.subtract,
                op1=mybir.AluOpType.add,
            ).then_inc(dve_sem, 1)
            eng.wait_ge(pe_sem, P_ZB)
            eng.wait_ge(dve_sem, V_RL2)
            eng.tensor_sub(zt_sb[:], affT_sb[:], ps_aff[:, 0:TOK]).then_inc(dve_sem, 1)

        # ---------------- gpsimd: collectives ----------------
        @block.gpsimd
        def _(eng):
            for it in range(ITERS - 1):
                eng.wait_ge(dma_sem, 16 * (2 * it + 1))
                eng.collective_compute(
                    "AllReduce",
                    mybir.AluOpType.add,
                    ins=[cc_in[:]],
                    outs=[cc_out[:]],
                    replica_groups=[core_ids],
                ).then_inc(cc_sem, 1)

    return nc


_CACHE = {}


def _get_nc():
    if "nc" not in _CACHE:
        _CACHE["nc"] = _build_nc()
    return _CACHE["nc"]


def make_in_maps(input_features, expert_centroids):
    feats = np.ascontiguousarray(np.asarray(input_features, dtype=np.float32).reshape(-1, D))
    cent = np.asarray(expert_centroids, dtype=np.float32).reshape(SE, D)

    featsT = np.ascontiguousarray(feats.T)
    centT = np.ascontiguousarray(cent.T)
    ident = np.eye(128, dtype=np.float32)
    ones = np.ones((1, 64), dtype=np.float32)
    onesc = np.ones((128, 1), dtype=np.float32)
    v0 = np.zeros((SE, 2), np.float32)
    v0[0:64, 0] = 1.0
    v0[64:128, 1] = 1.0

    in_maps = []
    for c in range(N_CORES):
        in_maps.append(
            {
                "featsT": np.ascontiguousarray(featsT[:, TOK * c : TOK * (c + 1)]),
                "centT": centT,
                "ident": ident,
                "ones": ones,
                "onesc": onesc,
                "v0": v0,
            }
        )
    return in_maps


def kernel(input_features: np.ndarray, expert_centroids: np.ndarray):
    in_maps = make_in_maps(input_features, expert_centroids)
    nc = _get_nc()
    res = run_bass_kernel_spmd(nc, in_maps, list(range(N_CORES)))

    zt = np.concatenate([res.results[c]["zt"] for c in range(N_CORES)], axis=1)
    afft = np.concatenate([res.results[c]["afft"] for c in range(N_CORES)], axis=1)

    Z = zt.reshape(KSLOT, E, N)
    A = afft.reshape(KSLOT, E, N)
    idx = np.empty((KSLOT, E, CAP), np.int32)
    vals = np.empty((KSLOT, E, CAP), np.float32)
    for k in range(KSLOT):
        for e in range(E):
            col = Z[k, e]
            part = np.sort(np.argpartition(-col, CAP - 1)[:CAP])
            order = part[np.argsort(-col[part], kind="stable")]
            idx[k, e] = order.astype(np.int32)
            vals[k, e] = A[k, e, order]
    return idx, vals


# revision 28
# speedup vs baseline: 1.4222x; 1.4222x over previous
"""Trainium2 Bass kernel for nn_BaseLayerGate (MoE balanced routing).

8 NeuronCores, data-parallel over tokens:
  - Each core owns a 2048-token shard. Affinity matmul aff^T = centT.T @ featsT
    on the tensor engine (fp32): col-major aff^T [128 (2 slots x 64 experts), 2048].
  - Sinkhorn (10 iters) in reciprocal-potential form:
      R_sum[n]  = sum_se E0[n,se] * V[se]      (PE matvec, V = slot-masked 1/C_sum)
      C_sum[se] = sum_n  E0[n,se] * W[n]       (PE matvec accum, W = 1/R_sum)
    The token-direction sum is global: per-expert partials are exchanged with an
    AllGather of a [1,128] row (PE-transposed so both exchange DMAs are
    contiguous), summed on-chip. 10 R-steps, 9 C-steps/exchanges (the 10th
    C-step is a uniform per-column shift and cannot change top-k ordering).
  - Z^T = aff^T - ln(R_sum) broadcast (ACT Ln + one Newton step for the LUT),
    per-column ordering of Z equals the reference's final ordering.
"""

import numpy as np

import concourse.bass as bass
from concourse import mybir
from concourse.bass_utils import run_bass_kernel_spmd

N_CORES = 8
N = 16384
D = 1024
KSLOT = 2
E = 64
SE = KSLOT * E
CAP = N // E
TOK = N // N_CORES
ITERS = 10

F32 = mybir.dt.float32

# ---- semaphore schedule ----------------------------------------------------
# in_sem: centT(8) + v0 + ident + ones = 11 transfers; fsem[k]: featsT chunk k
# out_sem: afft + zt outputs
# dma_sem (all x16): exchange cc_in(it) -> 2it+1, gath(it) -> 2it+2; rflat +2
D_EXCH_END = 2 * (ITERS - 1)                 # 18
D_RFLAT = D_EXCH_END + 2                     # 20
# pe_sem: 1 aff | +16 transposes -> 17 | per iter: R -> 17+2it+1, C -> 17+2it+2
def P_R(it):
    return 17 + 2 * it + 1
def P_C(it):
    return 17 + 2 * it + 2
P_LAST_R = P_R(ITERS - 1)                     # 36
P_ZB = P_LAST_R + 3                           # 39
# act_sem: 1 aff-copy | 2 exp | e0tm copy t -> t+3 (-> 18) | Ln | texp | rT x2
A_E0TM = 18
A_LN = A_E0TM + 1                             # 19
A_TEXP = A_LN + 1                             # 20
A_RT = A_TEXP + 2                             # 22
# dve_sem: 1 affT copy | per it<9: W=2+2it, VU=3+2it | u, rl2, sub
def V_W(it):
    return 4 * it + 1
def V_EX(it):
    return 4 * it + 2
def V_RD(it):
    return 4 * it + 3
def V_VU(it):
    return 4 * it + 4
V_U = 4 * (ITERS - 1) + 1                     # 37
V_RL2 = V_U + 1                               # 38
V_SUB = V_RL2 + 1                             # 39


def _build_nc():
    nc = bass.Bass()

    featsT_in = nc.declare_dram_parameter("featsT", [D, TOK], F32, isOutput=False)
    centT_in = nc.declare_dram_parameter("centT", [D, SE], F32, isOutput=False)
    v0_in = nc.declare_dram_parameter("v0", [SE, 2], F32, isOutput=False)
    ident_in = nc.declare_dram_parameter("ident", [128, 128], F32, isOutput=False)
    ones_in = nc.declare_dram_parameter("ones", [1, 64], F32, isOutput=False)
    onesc_in = nc.declare_dram_parameter("onesc", [128, 1], F32, isOutput=False)

    zt_out = nc.declare_dram_parameter("zt", [SE, TOK], F32, isOutput=True)
    aff_out = nc.declare_dram_parameter("afft", [SE, TOK], F32, isOutput=True)

    cc_in = nc.dram_tensor("cc_in", [SE, 1], F32)
    cc_out = nc.dram_tensor("cc_out", [N_CORES * SE, 1], F32, addr_space="Shared")

    core_ids = list(range(N_CORES))

    from contextlib import ExitStack
    es = ExitStack()
    featsT_sb = es.enter_context(nc.sbuf_tensor("featsT_sb", [128, 8, TOK], F32))
    centT_sb = es.enter_context(nc.sbuf_tensor("centT_sb", [128, 8, SE], F32))
    affT_sb = es.enter_context(nc.sbuf_tensor("affT_sb", [128, TOK], F32))
    e0t_sb = es.enter_context(nc.sbuf_tensor("e0t_sb", [128, TOK], F32))
    e0tm_sb = es.enter_context(nc.sbuf_tensor("e0tm_sb", [128, 16, 128], F32))
    ident_sb = es.enter_context(nc.sbuf_tensor("ident_sb", [128, 128], F32))
    v_sb = es.enter_context(nc.sbuf_tensor("v_sb", [128, 2], F32))
    w_sb = es.enter_context(nc.sbuf_tensor("w_sb", [128, 16, 2], F32))
    cpart_sb = es.enter_context(nc.sbuf_tensor("cpart_sb", [128, 1], F32))
    crow_sb = es.enter_context(nc.sbuf_tensor("crow_sb", [1, SE], F32))
    gath_sb = es.enter_context(nc.sbuf_tensor("gath_sb", [128, SE], F32))
    csum_sb = es.enter_context(nc.sbuf_tensor("csum_sb", [128, 1], F32))
    g8_sb = es.enter_context(nc.sbuf_tensor("g8_sb", [128, 8], F32))
    rlog_sb = es.enter_context(nc.sbuf_tensor("rlog_sb", [128, 16, 2], F32))
    texp_sb = es.enter_context(nc.sbuf_tensor("texp_sb", [128, 32], F32))
    u_sb = es.enter_context(nc.sbuf_tensor("u_sb", [128, 32], F32))
    rlog2_sb = es.enter_context(nc.sbuf_tensor("rlog2_sb", [128, 16, 2], F32))
    rt_sb = es.enter_context(nc.sbuf_tensor("rt_sb", [16, 2, 128], F32))
    rflat0_sb = es.enter_context(nc.sbuf_tensor("rflat0_sb", [1, TOK], F32))
    rflat1_sb = es.enter_context(nc.sbuf_tensor("rflat1_sb", [1, TOK], F32))
    ones_sb = es.enter_context(nc.sbuf_tensor("ones_sb", [1, 64], F32))
    onesc_sb = es.enter_context(nc.sbuf_tensor("onesc_sb", [128, 1], F32))
    zt_sb = es.enter_context(nc.sbuf_tensor("zt_sb", [128, TOK], F32))
    ps_aff = es.enter_context(nc.psum_tensor("ps_aff", [128, TOK], F32))
    ps_tp = es.enter_context(nc.psum_tensor("ps_tp", [128, 512], F32))
    ps_r = es.enter_context(nc.psum_tensor("ps_r", [128, 512], F32))
    ps_c = es.enter_context(nc.psum_tensor("ps_c", [128, 512], F32))
    block = es.enter_context(nc.Block())
    dma_sem = es.enter_context(nc.semaphore("dma_sem"))
    in_sem = es.enter_context(nc.semaphore("in_sem"))
    out_sem = es.enter_context(nc.semaphore("out_sem"))
    fsems = [es.enter_context(nc.semaphore(f"fsem{k}")) for k in range(8)]
    pe_sem = es.enter_context(nc.semaphore("pe_sem"))
    act_sem = es.enter_context(nc.semaphore("act_sem"))
    dve_sem = es.enter_context(nc.semaphore("dve_sem"))
    cc_sem = es.enter_context(nc.semaphore("cc_sem"))
    with es:
        # ---------------- sync engine: all DMA ----------------
        @block.sync
        def _(eng):
            for k in range(8):
                eng.dma_start(
                    out=centT_sb[:, k, :], in_=centT_in[128 * k : 128 * (k + 1), :]
                ).then_inc(in_sem, 16)
            eng.dma_start(out=v_sb[:], in_=v0_in[:]).then_inc(in_sem, 16)
            eng.dma_start(out=ident_sb[:], in_=ident_in[:]).then_inc(in_sem, 16)
            eng.dma_start(out=ones_sb[:], in_=ones_in[:]).then_inc(in_sem, 16)
            eng.dma_start(out=onesc_sb[:], in_=onesc_in[:]).then_inc(in_sem, 16)
            for k in range(8):
                eng.dma_start(
                    out=featsT_sb[:, k, :], in_=featsT_in[128 * k : 128 * (k + 1), :]
                ).then_inc(fsems[k], 16)

            eng.wait_ge(act_sem, 1)
            eng.dma_start(out=aff_out[:], in_=affT_sb[:]).then_inc(out_sem, 16)

            for it in range(ITERS - 1):
                eng.wait_ge(dve_sem, V_EX(it))
                eng.dma_start(out=cc_in[:], in_=cpart_sb[:]).then_inc(dma_sem, 16)
                eng.wait_ge(cc_sem, it + 1)
                src_ap = cc_out.ap().rearrange("(r e) o -> e (r o)", r=N_CORES)
                with nc.allow_non_contiguous_dma(reason="8x4B strided rank gather per partition"):
                    eng.dma_start(out=g8_sb[:], in_=src_ap).then_inc(dma_sem, 16)

            eng.wait_ge(act_sem, A_RT)
            for s in range(2):
                dsts = (rflat0_sb if s == 0 else rflat1_sb).ap()[0:1]
                dsts = dsts.rearrange("o (t p) -> o t p", p=128)
                eng.dma_start(out=dsts, in_=rt_sb[:, s, :]).then_inc(dma_sem, 16)

            eng.wait_ge(dve_sem, V_SUB)
            eng.dma_start(out=zt_out[:], in_=zt_sb[:]).then_inc(out_sem, 16)
            eng.wait_ge(out_sem, 32)
            eng.wait_ge(dma_sem, 16 * D_RFLAT)

        # ---------------- tensor engine ----------------
        @block.tensor
        def _(eng):
            eng.wait_ge(in_sem, 16 * 12)
            for k in range(8):
                eng.wait_ge(fsems[k], 16)
                for n in range(4):
                    mm = eng.matmul(
                        ps_aff[:, 512 * n : 512 * (n + 1)],
                        centT_sb[:, k, :],
                        featsT_sb[:, k, 512 * n : 512 * (n + 1)],
                        start=(k == 0),
                        stop=(k == 7),
                    )
            mm.then_inc(pe_sem, 1)

            for t in range(16):
                eng.wait_ge(act_sem, 2 + t)  # exp done (t=0) / ps_tp freed (t>0)
                eng.transpose(
                    ps_tp[:, 0:128], e0t_sb[:, 128 * t : 128 * (t + 1)], ident_sb[:]
                ).then_inc(pe_sem, 1)

            for it in range(ITERS):
                if it > 0:
                    eng.wait_ge(dve_sem, V_VU(it - 1))
                for t in range(16):
                    mm = eng.matmul(
                        ps_r[:, 2 * t : 2 * (t + 1)],
                        e0t_sb[:, 128 * t : 128 * (t + 1)],
                        v_sb[:],
                        start=True,
                        stop=True,
                    )
                mm.then_inc(pe_sem, 1)

                if it < ITERS - 1:
                    if it == 0:
                        eng.wait_ge(act_sem, A_E0TM)  # all e0tm copies landed
                    eng.wait_ge(dve_sem, V_W(it))
                    for t in range(16):
                        mm = eng.matmul(
                            ps_c[:, 0:2],
                            e0tm_sb[:, t, :],
                            w_sb[:, t, :],
                            start=(t == 0),
                            stop=(t == 15),
                        )
                    mm.then_inc(pe_sem, 1)

            eng.wait_ge(dve_sem, V_RL2)
            for s in range(2):
                eng.transpose(ps_tp[0:16, 0:128], rlog2_sb[:, :, s], ident_sb[:]).then_inc(pe_sem, 1)
                eng.wait_ge(act_sem, A_TEXP + 1 + s)  # ACT copied ps_tp before reuse

            eng.wait_ge(dma_sem, 16 * D_RFLAT)
            for s in range(2):
                rsrc = rflat0_sb if s == 0 else rflat1_sb
                for n in range(4):
                    mm = eng.matmul(
                        ps_aff[64 * s : 64 * (s + 1), 512 * n : 512 * (n + 1)],
                        ones_sb[0:1, :],
                        rsrc[0:1, 512 * n : 512 * (n + 1)],
                        start=True,
                        stop=True,
                    )
            mm.then_inc(pe_sem, 1)

        # ---------------- scalar (ACT) engine ----------------
        @block.scalar
        def _(eng):
            eng.wait_ge(pe_sem, 1)
            eng.activation(affT_sb[:], ps_aff[:, 0:TOK], mybir.ActivationFunctionType.Copy).then_inc(act_sem, 1)
            eng.wait_ge(act_sem, 1)
            eng.activation(e0t_sb[:], affT_sb[:], mybir.ActivationFunctionType.Exp).then_inc(act_sem, 1)
            for t in range(16):
                eng.wait_ge(pe_sem, 2 + t)
                eng.activation(
                    e0tm_sb[:, t, :], ps_tp[:, 0:128], mybir.ActivationFunctionType.Copy
                ).then_inc(act_sem, 1)
            eng.wait_ge(pe_sem, P_LAST_R)
            eng.activation(
                rlog_sb.ap().rearrange("p t s -> p (t s)"),
                ps_r[:, 0:32],
                mybir.ActivationFunctionType.Ln,
            ).then_inc(act_sem, 1)
            eng.wait_ge(act_sem, A_LN)
            eng.activation(
                texp_sb[:],
                rlog_sb.ap().rearrange("p t s -> p (t s)"),
                mybir.ActivationFunctionType.Exp,
                scale=-1.0,
            ).then_inc(act_sem, 1)
            for s in range(2):
                eng.wait_ge(pe_sem, P_LAST_R + 1 + s)
                eng.activation(rt_sb[:, s, :], ps_tp[0:16, 0:128], mybir.ActivationFunctionType.Copy).then_inc(act_sem, 1)


        # ---------------- vector (DVE) engine ----------------
        @block.vector
        def _(eng):
            for it in range(ITERS - 1):
                eng.wait_ge(pe_sem, P_R(it))
                eng.reciprocal(w_sb.ap().rearrange("p t s -> p (t s)"), ps_r[:, 0:32]).then_inc(dve_sem, 1)
                eng.wait_ge(pe_sem, P_C(it))
                eng.tensor_copy(cpart_sb[0:64, :], ps_c[0:64, 0:1])
                eng.tensor_copy(cpart_sb[64:128, :], ps_c[64:128, 1:2]).then_inc(dve_sem, 1)
                eng.wait_ge(dma_sem, 16 * (2 * it + 2))
                eng.tensor_reduce(
                    csum_sb[:], g8_sb[:], mybir.AxisListType.X, mybir.AluOpType.add
                ).then_inc(dve_sem, 1)
                eng.wait_ge(dve_sem, V_RD(it))
                eng.reciprocal(v_sb[0:64, 0:1], csum_sb[0:64, :])
                eng.reciprocal(v_sb[64:128, 1:2], csum_sb[64:128, :]).then_inc(dve_sem, 1)
            eng.wait_ge(pe_sem, P_LAST_R)
            eng.wait_ge(act_sem, A_TEXP)
            eng.tensor_mul(u_sb[:], ps_r[:, 0:32], texp_sb[:]).then_inc(dve_sem, 1)
            eng.wait_ge(dve_sem, V_U)
            eng.scalar_tensor_tensor(
                rlog2_sb.ap().rearrange("p t s -> p (t s)"),
                u_sb[:],
                1.0,
                rlog_sb.ap().rearrange("p t s -> p (t s)"),
                op0=mybir.AluOpType.subtract,
                op1=mybir.AluOpType.add,
            ).then_inc(dve_sem, 1)
            eng.wait_ge(pe_sem, P_ZB)
            eng.wait_ge(dve_sem, V_RL2)
            eng.tensor_sub(zt_sb[:], affT_sb[:], ps_aff[:, 0:TOK]).then_inc(dve_sem, 1)

        # ---------------- gpsimd: collectives ----------------
        @block.gpsimd
        def _(eng):
            for it in range(ITERS - 1):
                eng.wait_ge(dma_sem, 16 * (2 * it + 1))
                eng.collective_compute(
                    "AllGather",
                    mybir.AluOpType.bypass,
                    ins=[cc_in[:]],
                    outs=[cc_out[:]],
                    replica_groups=[core_ids],
                ).then_inc(cc_sem, 1)

    return nc


_CACHE = {}


def _get_nc():
    if "nc" not in _CACHE:
        _CACHE["nc"] = _build_nc()
    return _CACHE["nc"]


def make_in_maps(input_features, expert_centroids):
    feats = np.ascontiguousarray(np.asarray(input_features, dtype=np.float32).reshape(-1, D))
    cent = np.asarray(expert_centroids, dtype=np.float32).reshape(SE, D)

    featsT = np.ascontiguousarray(feats.T)
    centT = np.ascontiguousarray(cent.T)
    ident = np.eye(128, dtype=np.float32)
    ones = np.ones((1, 64), dtype=np.float32)
    onesc = np.ones((128, 1), dtype=np.float32)
    v0 = np.zeros((SE, 2), np.float32)
    v0[0:64, 0] = 1.0
    v0[64:128, 1] = 1.0

    in_maps = []
    for c in range(N_CORES):
        in_maps.append(
            {
                "featsT": np.ascontiguousarray(featsT[:, TOK * c : TOK * (c + 1)]),
                "centT": centT,
                "ident": ident,
                "ones": ones,
                "onesc": onesc,
                "v0": v0,
            }
        )
    return in_maps


def kernel(input_features: np.ndarray, expert_centroids: np.ndarray):
    in_maps = make_in_maps(input_features, expert_centroids)
    nc = _get_nc()
    res = run_bass_kernel_spmd(nc, in_maps, list(range(N_CORES)))

    zt = np.concatenate([res.results[c]["zt"] for c in range(N_CORES)], axis=1)
    afft = np.concatenate([res.results[c]["afft"] for c in range(N_CORES)], axis=1)

    Z = zt.reshape(KSLOT, E, N)
    A = afft.reshape(KSLOT, E, N)
    idx = np.empty((KSLOT, E, CAP), np.int32)
    vals = np.empty((KSLOT, E, CAP), np.float32)
    for k in range(KSLOT):
        for e in range(E):
            col = Z[k, e]
            part = np.sort(np.argpartition(-col, CAP - 1)[:CAP])
            order = part[np.argsort(-col[part], kind="stable")]
            idx[k, e] = order.astype(np.int32)
            vals[k, e] = A[k, e, order]
    return idx, vals


# revision 31
# speedup vs baseline: 1.4456x; 1.0165x over previous
"""Trainium2 Bass kernel for nn_BaseLayerGate (MoE balanced routing).

8 NeuronCores, data-parallel over tokens:
  - Each core owns a 2048-token shard. Affinity matmul aff^T = centT.T @ featsT
    on the tensor engine (fp32): col-major aff^T [128 (2 slots x 64 experts), 2048].
  - Sinkhorn (10 iters) in reciprocal-potential form:
      R_sum[n]  = sum_se E0[n,se] * V[se]      (PE matvec, V = slot-masked 1/C_sum)
      C_sum[se] = sum_n  E0[n,se] * W[n]       (PE matvec accum, W = 1/R_sum)
    The token-direction sum is global: per-expert partials are exchanged with an
    AllGather of a [1,128] row (PE-transposed so both exchange DMAs are
    contiguous), summed on-chip. 10 R-steps, 9 C-steps/exchanges (the 10th
    C-step is a uniform per-column shift and cannot change top-k ordering).
  - Z^T = aff^T - ln(R_sum) broadcast (ACT Ln + one Newton step for the LUT),
    per-column ordering of Z equals the reference's final ordering.
"""

import numpy as np

import concourse.bass as bass
from concourse import mybir
from concourse.bass_utils import run_bass_kernel_spmd

N_CORES = 8
N = 16384
D = 1024
KSLOT = 2
E = 64
SE = KSLOT * E
CAP = N // E
TOK = N // N_CORES
ITERS = 10

F32 = mybir.dt.float32

# ---- semaphore schedule ----------------------------------------------------
# in_sem: centT(8) + v0 + ident + ones = 11 transfers; fsem[k]: featsT chunk k
# out_sem: afft + zt outputs
# dma_sem (all x16): exchange cc_in(it) -> 2it+1, gath(it) -> 2it+2; rflat +2
D_EXCH_END = 2 * (ITERS - 1)                 # 18
D_RFLAT = D_EXCH_END + 2                     # 20
# pe_sem: 1 aff | 2 R(0) | 3..18 transposes | C(it)=19+2it, R(it>=1)=18+2it
def P_R(it):
    return 2 if it == 0 else 18 + 2 * it
def P_C(it):
    return 19 + 2 * it
P_LAST_R = P_R(ITERS - 1)                     # 36
P_ZB = P_LAST_R + 3                           # 39
# act_sem: 1 aff-copy | 2 exp | e0tm copy t -> t+3 (-> 18) | Ln | texp | rT x2
A_E0TM = 18
A_LN = A_E0TM + 1                             # 19
A_TEXP = A_LN + 1                             # 20
A_RT = A_TEXP + 2                             # 22
# dve_sem: 1 affT copy | per it<9: W=2+2it, VU=3+2it | u, rl2, sub
def V_W(it):
    return 4 * it + 1
def V_EX(it):
    return 4 * it + 2
def V_RD(it):
    return 4 * it + 3
def V_VU(it):
    return 4 * it + 4
V_U = 4 * (ITERS - 1) + 1                     # 37
V_RL2 = V_U + 1                               # 38
V_SUB = V_RL2 + 1                             # 39


def _build_nc():
    nc = bass.Bass()

    featsT_in = nc.declare_dram_parameter("featsT", [D, TOK], F32, isOutput=False)
    centT_in = nc.declare_dram_parameter("centT", [D, SE], F32, isOutput=False)
    v0_in = nc.declare_dram_parameter("v0", [SE, 2], F32, isOutput=False)
    ident_in = nc.declare_dram_parameter("ident", [128, 128], F32, isOutput=False)
    ones_in = nc.declare_dram_parameter("ones", [1, 64], F32, isOutput=False)
    onesc_in = nc.declare_dram_parameter("onesc", [128, 1], F32, isOutput=False)

    zt_out = nc.declare_dram_parameter("zt", [SE, TOK], F32, isOutput=True)
    aff_out = nc.declare_dram_parameter("afft", [SE, TOK], F32, isOutput=True)

    cc_in = nc.dram_tensor("cc_in", [SE, 1], F32)
    cc_out = nc.dram_tensor("cc_out", [N_CORES * SE, 1], F32, addr_space="Shared")

    core_ids = list(range(N_CORES))

    from contextlib import ExitStack
    es = ExitStack()
    featsT_sb = es.enter_context(nc.sbuf_tensor("featsT_sb", [128, 8, TOK], F32))
    centT_sb = es.enter_context(nc.sbuf_tensor("centT_sb", [128, 8, SE], F32))
    affT_sb = es.enter_context(nc.sbuf_tensor("affT_sb", [128, TOK], F32))
    e0t_sb = es.enter_context(nc.sbuf_tensor("e0t_sb", [128, TOK], F32))
    e0tm_sb = es.enter_context(nc.sbuf_tensor("e0tm_sb", [128, 16, 128], F32))
    ident_sb = es.enter_context(nc.sbuf_tensor("ident_sb", [128, 128], F32))
    v_sb = es.enter_context(nc.sbuf_tensor("v_sb", [128, 2], F32))
    w_sb = es.enter_context(nc.sbuf_tensor("w_sb", [128, 16, 2], F32))
    cpart_sb = es.enter_context(nc.sbuf_tensor("cpart_sb", [128, 1], F32))
    crow_sb = es.enter_context(nc.sbuf_tensor("crow_sb", [1, SE], F32))
    gath_sb = es.enter_context(nc.sbuf_tensor("gath_sb", [128, SE], F32))
    csum_sb = es.enter_context(nc.sbuf_tensor("csum_sb", [128, 1], F32))
    g8_sb = es.enter_context(nc.sbuf_tensor("g8_sb", [128, 8], F32))
    rlog_sb = es.enter_context(nc.sbuf_tensor("rlog_sb", [128, 16, 2], F32))
    texp_sb = es.enter_context(nc.sbuf_tensor("texp_sb", [128, 32], F32))
    u_sb = es.enter_context(nc.sbuf_tensor("u_sb", [128, 32], F32))
    rlog2_sb = es.enter_context(nc.sbuf_tensor("rlog2_sb", [128, 16, 2], F32))
    rt_sb = es.enter_context(nc.sbuf_tensor("rt_sb", [16, 2, 128], F32))
    rflat0_sb = es.enter_context(nc.sbuf_tensor("rflat0_sb", [1, TOK], F32))
    rflat1_sb = es.enter_context(nc.sbuf_tensor("rflat1_sb", [1, TOK], F32))
    ones_sb = es.enter_context(nc.sbuf_tensor("ones_sb", [1, 64], F32))
    onesc_sb = es.enter_context(nc.sbuf_tensor("onesc_sb", [128, 1], F32))
    zt_sb = es.enter_context(nc.sbuf_tensor("zt_sb", [128, TOK], F32))
    ps_aff = es.enter_context(nc.psum_tensor("ps_aff", [128, TOK], F32))
    ps_tp = es.enter_context(nc.psum_tensor("ps_tp", [128, 512], F32))
    ps_r = es.enter_context(nc.psum_tensor("ps_r", [128, 512], F32))
    ps_c = es.enter_context(nc.psum_tensor("ps_c", [128, 512], F32))
    ps_tp2 = es.enter_context(nc.psum_tensor("ps_tp2", [128, 512], F32))
    block = es.enter_context(nc.Block())
    dma_sem = es.enter_context(nc.semaphore("dma_sem"))
    in_sem = es.enter_context(nc.semaphore("in_sem"))
    out_sem = es.enter_context(nc.semaphore("out_sem"))
    fsems = [es.enter_context(nc.semaphore(f"fsem{k}")) for k in range(8)]
    pe_sem = es.enter_context(nc.semaphore("pe_sem"))
    act_sem = es.enter_context(nc.semaphore("act_sem"))
    dve_sem = es.enter_context(nc.semaphore("dve_sem"))
    cc_sem = es.enter_context(nc.semaphore("cc_sem"))
    with es:
        # ---------------- sync engine: all DMA ----------------
        @block.sync
        def _(eng):
            for k in range(8):
                eng.dma_start(
                    out=centT_sb[:, k, :], in_=centT_in[128 * k : 128 * (k + 1), :]
                ).then_inc(in_sem, 16)
            eng.dma_start(out=v_sb[:], in_=v0_in[:]).then_inc(in_sem, 16)
            eng.dma_start(out=ident_sb[:], in_=ident_in[:]).then_inc(in_sem, 16)
            eng.dma_start(out=ones_sb[:], in_=ones_in[:]).then_inc(in_sem, 16)
            eng.dma_start(out=onesc_sb[:], in_=onesc_in[:]).then_inc(in_sem, 16)
            for k in range(8):
                eng.dma_start(
                    out=featsT_sb[:, k, :], in_=featsT_in[128 * k : 128 * (k + 1), :]
                ).then_inc(fsems[k], 16)

            eng.wait_ge(act_sem, 1)
            eng.dma_start(out=aff_out[:], in_=affT_sb[:]).then_inc(out_sem, 16)

            for it in range(ITERS - 1):
                eng.wait_ge(dve_sem, V_EX(it))
                eng.dma_start(out=cc_in[:], in_=cpart_sb[:]).then_inc(dma_sem, 16)
                eng.wait_ge(cc_sem, it + 1)
                src_ap = cc_out.ap().rearrange("(r e) o -> e (r o)", r=N_CORES)
                with nc.allow_non_contiguous_dma(reason="8x4B strided rank gather per partition"):
                    eng.dma_start(out=g8_sb[:], in_=src_ap).then_inc(dma_sem, 16)

            eng.wait_ge(act_sem, A_RT)
            for s in range(2):
                dsts = (rflat0_sb if s == 0 else rflat1_sb).ap()[0:1]
                dsts = dsts.rearrange("o (t p) -> o t p", p=128)
                eng.dma_start(out=dsts, in_=rt_sb[:, s, :]).then_inc(dma_sem, 16)

            eng.wait_ge(dve_sem, V_SUB)
            eng.dma_start(out=zt_out[:], in_=zt_sb[:]).then_inc(out_sem, 16)
            eng.wait_ge(out_sem, 32)
            eng.wait_ge(dma_sem, 16 * D_RFLAT)

        # ---------------- tensor engine ----------------
        @block.tensor
        def _(eng):
            eng.wait_ge(in_sem, 16 * 12)
            for k in range(8):
                eng.wait_ge(fsems[k], 16)
                for n in range(4):
                    mm = eng.matmul(
                        ps_aff[:, 512 * n : 512 * (n + 1)],
                        centT_sb[:, k, :],
                        featsT_sb[:, k, 512 * n : 512 * (n + 1)],
                        start=(k == 0),
                        stop=(k == 7),
                    )
            mm.then_inc(pe_sem, 1)

            # iteration-0 R-step right after exp (E0_tm not needed for it)
            eng.wait_ge(act_sem, 2)
            for t in range(16):
                mm = eng.matmul(
                    ps_r[:, 2 * t : 2 * (t + 1)],
                    e0t_sb[:, 128 * t : 128 * (t + 1)],
                    v_sb[:],
                    start=True,
                    stop=True,
                )
            mm.then_inc(pe_sem, 1)

            for t in range(16):
                if t >= 2:
                    eng.wait_ge(act_sem, t + 1)  # copy t-2 freed this buffer
                buf = ps_tp if t % 2 == 0 else ps_tp2
                eng.transpose(
                    buf[:, 0:128], e0t_sb[:, 128 * t : 128 * (t + 1)], ident_sb[:]
                ).then_inc(pe_sem, 1)

            for it in range(ITERS):
                if it > 0:
                    eng.wait_ge(dve_sem, V_VU(it - 1))
                    for t in range(16):
                        mm = eng.matmul(
                            ps_r[:, 2 * t : 2 * (t + 1)],
                            e0t_sb[:, 128 * t : 128 * (t + 1)],
                            v_sb[:],
                            start=True,
                            stop=True,
                        )
                    mm.then_inc(pe_sem, 1)

                if it < ITERS - 1:
                    if it == 0:
                        eng.wait_ge(act_sem, A_E0TM)  # all e0tm copies landed
                    eng.wait_ge(dve_sem, V_W(it))
                    for t in range(16):
                        mm = eng.matmul(
                            ps_c[:, 0:2],
                            e0tm_sb[:, t, :],
                            w_sb[:, t, :],
                            start=(t == 0),
                            stop=(t == 15),
                        )
                    mm.then_inc(pe_sem, 1)

            eng.wait_ge(dve_sem, V_RL2)
            for s in range(2):
                eng.transpose(ps_tp[0:16, 0:128], rlog2_sb[:, :, s], ident_sb[:]).then_inc(pe_sem, 1)
                eng.wait_ge(act_sem, A_TEXP + 1 + s)  # ACT copied ps_tp before reuse

            eng.wait_ge(dma_sem, 16 * D_RFLAT)
            for s in range(2):
                rsrc = rflat0_sb if s == 0 else rflat1_sb
                for n in range(4):
                    mm = eng.matmul(
                        ps_aff[64 * s : 64 * (s + 1), 512 * n : 512 * (n + 1)],
                        ones_sb[0:1, :],
                        rsrc[0:1, 512 * n : 512 * (n + 1)],
                        start=True,
                        stop=True,
                    )
            mm.then_inc(pe_sem, 1)

        # ---------------- scalar (ACT) engine ----------------
        @block.scalar
        def _(eng):
            eng.wait_ge(pe_sem, 1)
            eng.activation(affT_sb[:], ps_aff[:, 0:TOK], mybir.ActivationFunctionType.Copy).then_inc(act_sem, 1)
            eng.wait_ge(act_sem, 1)
            eng.activation(e0t_sb[:], affT_sb[:], mybir.ActivationFunctionType.Exp).then_inc(act_sem, 1)
            for t in range(16):
                eng.wait_ge(pe_sem, 3 + t)
                buf = ps_tp if t % 2 == 0 else ps_tp2
                eng.activation(
                    e0tm_sb[:, t, :], buf[:, 0:128], mybir.ActivationFunctionType.Copy
                ).then_inc(act_sem, 1)
            eng.wait_ge(pe_sem, P_LAST_R)
            eng.activation(
                rlog_sb.ap().rearrange("p t s -> p (t s)"),
                ps_r[:, 0:32],
                mybir.ActivationFunctionType.Ln,
            ).then_inc(act_sem, 1)
            eng.wait_ge(act_sem, A_LN)
            eng.activation(
                texp_sb[:],
                rlog_sb.ap().rearrange("p t s -> p (t s)"),
                mybir.ActivationFunctionType.Exp,
                scale=-1.0,
            ).then_inc(act_sem, 1)
            for s in range(2):
                eng.wait_ge(pe_sem, P_LAST_R + 1 + s)
                eng.activation(rt_sb[:, s, :], ps_tp[0:16, 0:128], mybir.ActivationFunctionType.Copy).then_inc(act_sem, 1)


        # ---------------- vector (DVE) engine ----------------
        @block.vector
        def _(eng):
            for it in range(ITERS - 1):
                eng.wait_ge(pe_sem, P_R(it))
                eng.reciprocal(w_sb.ap().rearrange("p t s -> p (t s)"), ps_r[:, 0:32]).then_inc(dve_sem, 1)
                eng.wait_ge(pe_sem, P_C(it))
                eng.tensor_copy(cpart_sb[0:64, :], ps_c[0:64, 0:1])
                eng.tensor_copy(cpart_sb[64:128, :], ps_c[64:128, 1:2]).then_inc(dve_sem, 1)
                eng.wait_ge(dma_sem, 16 * (2 * it + 2))
                eng.tensor_reduce(
                    csum_sb[:], g8_sb[:], mybir.AxisListType.X, mybir.AluOpType.add
                ).then_inc(dve_sem, 1)
                eng.wait_ge(dve_sem, V_RD(it))
                eng.reciprocal(v_sb[0:64, 0:1], csum_sb[0:64, :])
                eng.reciprocal(v_sb[64:128, 1:2], csum_sb[64:128, :]).then_inc(dve_sem, 1)
            eng.wait_ge(pe_sem, P_LAST_R)
            eng.wait_ge(act_sem, A_TEXP)
            eng.tensor_mul(u_sb[:], ps_r[:, 0:32], texp_sb[:]).then_inc(dve_sem, 1)
            eng.wait_ge(dve_sem, V_U)
            eng.scalar_tensor_tensor(
                rlog2_sb.ap().rearrange("p t s -> p (t s)"),
                u_sb[:],
                1.0,
                rlog_sb.ap().rearrange("p t s -> p (t s)"),
                op0=mybir.AluOpType.subtract,
                op1=mybir.AluOpType.add,
            ).then_inc(dve_sem, 1)
            eng.wait_ge(pe_sem, P_ZB)
            eng.wait_ge(dve_sem, V_RL2)
            eng.tensor_sub(zt_sb[:], affT_sb[:], ps_aff[:, 0:TOK]).then_inc(dve_sem, 1)

        # ---------------- gpsimd: collectives ----------------
        @block.gpsimd
        def _(eng):
            for it in range(ITERS - 1):
                eng.wait_ge(dma_sem, 16 * (2 * it + 1))
                eng.collective_compute(
                    "AllGather",
                    mybir.AluOpType.bypass,
                    ins=[cc_in[:]],
                    outs=[cc_out[:]],
                    replica_groups=[core_ids],
                ).then_inc(cc_sem, 1)

    return nc


_CACHE = {}


def _get_nc():
    if "nc" not in _CACHE:
        _CACHE["nc"] = _build_nc()
    return _CACHE["nc"]


def make_in_maps(input_features, expert_centroids):
    feats = np.ascontiguousarray(np.asarray(input_features, dtype=np.float32).reshape(-1, D))
    cent = np.asarray(expert_centroids, dtype=np.float32).reshape(SE, D)

    featsT = np.ascontiguousarray(feats.T)
    centT = np.ascontiguousarray(cent.T)
    ident = np.eye(128, dtype=np.float32)
    ones = np.ones((1, 64), dtype=np.float32)
    onesc = np.ones((128, 1), dtype=np.float32)
    v0 = np.zeros((SE, 2), np.float32)
    v0[0:64, 0] = 1.0
    v0[64:128, 1] = 1.0

    in_maps = []
    for c in range(N_CORES):
        in_maps.append(
            {
                "featsT": np.ascontiguousarray(featsT[:, TOK * c : TOK * (c + 1)]),
                "centT": centT,
                "ident": ident,
                "ones": ones,
                "onesc": onesc,
                "v0": v0,
            }
        )
    return in_maps


def kernel(input_features: np.ndarray, expert_centroids: np.ndarray):
    in_maps = make_in_maps(input_features, expert_centroids)
    nc = _get_nc()
    res = run_bass_kernel_spmd(nc, in_maps, list(range(N_CORES)))

    zt = np.concatenate([res.results[c]["zt"] for c in range(N_CORES)], axis=1)
    afft = np.concatenate([res.results[c]["afft"] for c in range(N_CORES)], axis=1)

    Z = zt.reshape(KSLOT, E, N)
    A = afft.reshape(KSLOT, E, N)
    idx = np.empty((KSLOT, E, CAP), np.int32)
    vals = np.empty((KSLOT, E, CAP), np.float32)
    for k in range(KSLOT):
        for e in range(E):
            col = Z[k, e]
            part = np.sort(np.argpartition(-col, CAP - 1)[:CAP])
            order = part[np.argsort(-col[part], kind="stable")]
            idx[k, e] = order.astype(np.int32)
            vals[k, e] = A[k, e, order]
    return idx, vals
